# revision 26
# baseline (speedup 1.0000x reference)
"""DGI (Deep Graph Infomax) kernel for 8 Trainium2 NeuronCores.

Strategy (row-wise graph partitioning + cross-iteration software pipeline):
  - nodes split 12500/core (padded to 12544 = 98*128 rows); each core owns
    the incoming edges of its node block.
  - phase 1: each core computes its shard of xtheta = x @ W^T + b for both
    graphs in row layout, rows stored as [node, pos_h(64) | neg_h(64)] bf16
    (256B rows). Stores are batched: one strided DMA per 8-group slab
    (DRAM AP transposed "t p c -> p t c") instead of 98 x 32KB stores.
  - the AllGather is split into 4 row-chunk collectives (3136 rows/chunk):
    AG_k concatenates chunk k of every rank -> xt_all_k [25088, 128].
    Gather buckets are keyed by SOURCE CHUNK (not rank pair), so each
    bucket's table is one contiguous xt_all_k and bucket-k gathers depend
    only on AG_k. Same block padding as rank-pair bucketing.
  - per-edge gather of 256B source rows via gpsimd dma_gather on 4 SWDGE
    queues (queue q <-> bucket q). Padding slots use spread indices (all-
    row-0 padding serializes on one HBM port); trailing padding of each
    call is -1 (ucode trims negative tail indices before descgen).
  - segment-sum via matmul: full 128-edge blocks target a 128-col slice of
    a [128, 512] PSUM tile (quad of 4 dest groups); the tails of the 4
    groups are merged into shared blocks with a 512-wide host-built M
    (arbitrary sparse rhs, not just one-hot), cutting gather slots ~10%.
  - PReLU + mean-readout row-sum fused into one Activation op per group.
  - cross-rep software pipeline: xt_c/xt_all/HT/acc/ar are double-buffered
    and emission is skewed [ph1(i+1); gathers(i) part A; AG(i+1) x4;
    gathers(i) part B; AR(i); ph3(i)] so collectives and phase 1/3 hide
    under the gather descriptor-generation wall (the serial resource).
"""

import sys

try:
    import concourse.bacc as bacc
except ImportError:  # pragma: no cover
    sys.path.insert(0, "/opt/trn_rl_repo")
    import concourse.bacc as bacc

import numpy as np
import ml_dtypes

import concourse.bass as bass
import concourse.mybir as mybir
import concourse.tile as tile
from concourse.library_config import mlp
from concourse.bass_utils import run_bass_kernel_spmd

P = 128
QUAD = 4            # dest groups merged per PSUM tile
GCHUNK = 8          # dest groups per gather/matmul chunk (2 quads)
NBUCKET = 4         # source chunks (= SWDGE queues)
BF16 = mybir.dt.bfloat16
F32 = mybir.dt.float32
I16 = mybir.dt.int16

_NC_CACHE = {}


# --------------------------------------------------------------------------
# host-side planning
# --------------------------------------------------------------------------

class Plan:
    pass


class Block:
    """One 128-slot gather/matmul block.

    kind 'full': 128 edges of one dest group  -> 128-wide mb slice
    kind 'tail': merged tail edges of a quad  -> (128*quad_len)-wide mb
    """
    __slots__ = ("kind", "g", "quad", "q", "pos", "mb_off", "mb_w",
                 "start", "stop", "slices", "_fill")

    def __init__(self, kind, g, quad, q):
        self.kind = kind
        self.g = g          # dest group (fulls) or first group of quad
        self.quad = quad
        self.q = q


def make_plan(n_nodes, ncores, edge_rows, edge_cols, edge_vals):
    pl = Plan()
    local_n = n_nodes // ncores
    assert local_n * ncores == n_nodes
    local_pad = ((local_n + P - 1) // P) * P
    groups = local_pad // P
    chunk_rows = local_pad // NBUCKET          # 3136
    assert chunk_rows * NBUCKET == local_pad
    bucket_rows = ncores * chunk_rows          # 25088 <= 32767
    assert bucket_rows <= 32767

    pl.ncores, pl.local_n, pl.local_pad = ncores, local_n, local_pad
    pl.groups, pl.chunk_rows, pl.bucket_rows = groups, chunk_rows, bucket_rows
    pl.trows = ncores * local_pad

    r = np.asarray(edge_rows).astype(np.int64)
    c = np.asarray(edge_cols).astype(np.int64)
    v = np.asarray(edge_vals).astype(np.float32)

    core = r // local_n
    lr = r % local_n
    g = lr // P
    crank = c // local_n
    cloc = c % local_n
    q = cloc // chunk_rows                      # source-chunk bucket
    idx16 = crank * chunk_rows + (cloc - q * chunk_rows)

    # quad structure over dest groups
    quads = [list(range(k, min(k + QUAD, groups))) for k in range(0, groups, QUAD)]
    pl.quads = quads
    quad_of_g = np.zeros(groups, np.int64)
    for qi, gs in enumerate(quads):
        for gg in gs:
            quad_of_g[gg] = qi

    # per (core, g, q) edge counts -> shared (max-over-core) block layout
    key = (core * groups + g) * NBUCKET + q
    counts = np.bincount(key, minlength=ncores * groups * NBUCKET).reshape(
        ncores, groups, NBUCKET)
    cmax = counts.max(axis=0)                  # [groups, NBUCKET]
    nfull = cmax // P                          # full blocks per (g, q)
    tail = cmax - nfull * P                    # tail edges per (g, q)
    # every group needs >=1 block writer per 128-col psum slice; if a group
    # has no full block in any q, promote its largest tail to a full block
    for gg in range(groups):
        if nfull[gg].sum() == 0:
            qq = int(np.argmax(tail[gg]))
            nfull[gg][qq] = 1
            tail[gg][qq] = 0
    # merged tail blocks per (quad, q)
    tailsum = np.zeros((len(quads), NBUCKET), np.int64)
    for qi, gs in enumerate(quads):
        for qq in range(NBUCKET):
            tailsum[qi][qq] = sum(int(tail[gg][qq]) for gg in gs)
    ntail = (tailsum + P - 1) // P

    # chunk structure (GCHUNK dest groups per chunk)
    chunk_gs = [list(range(k, min(k + GCHUNK, groups)))
                for k in range(0, groups, GCHUNK)]
    pl.chunks = []
    mb_off = 0          # bf16 cols into mbh (full blocks only)
    tail_off = 0        # tail-block counter (trl/tvv columns)
    idx_off = 0         # int16 cols into gidx
    jslot = 0           # global block slot counter
    for gs in chunk_gs:
        spec = Plan()
        spec.groups = gs
        spec.quads = sorted({int(quad_of_g[gg]) for gg in gs})
        spec.idx_off = idx_off
        spec.q_off = []
        spec.nq = []
        spec.blocks = []            # flat list of Block in gd-slot order
        for qq in range(NBUCKET):
            spec.q_off.append(idx_off - spec.idx_off)
            nq = 0
            for gg in gs:
                for _ in range(int(nfull[gg][qq])):
                    b = Block("full", gg, int(quad_of_g[gg]), qq)
                    b.pos = (qq, nq)
                    spec.blocks.append(b)
                    nq += 1
            for qi in spec.quads:
                for _ in range(int(ntail[qi][qq])):
                    b = Block("tail", pl.quads[qi][0], qi, qq)
                    b.pos = (qq, nq)
                    spec.blocks.append(b)
                    nq += 1
            spec.nq.append(nq)
            idx_off += nq * 8       # nq*128 idx -> /16 cols
        spec.idx_len = idx_off - spec.idx_off
        # mb layout: fulls host-streamed (128 cols each); tails DVE-built
        # from (trl, tvv) columns (one per tail block)
        spec.mb_off0 = mb_off
        for b in spec.blocks:
            b.mb_w = P if b.kind == "full" else P * len(pl.quads[b.quad])
            if b.kind == "full":
                b.mb_off = mb_off
                mb_off += P
            else:
                b.mb_off = tail_off
                tail_off += 1
        spec.mb_len = mb_off - spec.mb_off0
        jslot += sum(spec.nq)
        pl.chunks.append(spec)
    pl.b_total = jslot
    pl.gidx_cols = idx_off
    pl.mbh_cols = mb_off
    pl.ntails = tail_off

    # ---- matmul emission order + start/stop flags ----
    # PSUM accumulation groups are 2KB-bank granular: start=True marks the
    # whole bank pending-zero (first write per address overwrites, later
    # writes accumulate). So per quad tile: start on the FIRST matmul into
    # the tile, stop on the LAST, regardless of column slice.
    for spec in pl.chunks:
        fulls = [b for b in spec.blocks if b.kind == "full"]
        tails = [b for b in spec.blocks if b.kind == "tail"]
        spec.mm_order = fulls + tails
        first_q = {}
        last_q = {}
        for b in spec.mm_order:
            if b.quad not in first_q:
                first_q[b.quad] = b
            last_q[b.quad] = b
        for b in spec.mm_order:
            b.start = first_q[b.quad] is b
            b.stop = last_q[b.quad] is b

    # ---- per-core edge data ----
    order = np.lexsort((idx16, q, g, core))
    so_core, so_g, so_q = core[order], g[order], q[order]
    so_idx, so_lr, so_v = idx16[order], lr[order], v[order]
    seg_key = (so_core * groups + so_g) * NBUCKET + so_q
    seg_counts = np.bincount(seg_key, minlength=ncores * groups * NBUCKET)
    seg_starts = np.concatenate([[0], np.cumsum(seg_counts)])

    pl.gidx = []
    pl.mbh = []
    pl.trl = []
    pl.tvv = []
    rng = np.random.default_rng(1234)
    for cc in range(ncores):
        # per (g, q): this core's edges (idx, rloc, v), rloc relative to
        # the group's 128 rows
        def seg(gg, qq):
            sk = (cc * groups + gg) * NBUCKET + qq
            s0, s1 = seg_starts[sk], seg_starts[sk + 1]
            return (so_idx[s0:s1], (so_lr[s0:s1] - gg * P), so_v[s0:s1])

        # distribute edges to blocks: per (g,q) first nfull*P edges fill
        # the full blocks; the remainder goes to the quad's tail blocks.
        mbh = np.zeros((P, pl.mbh_cols), np.float32)
        trl = np.zeros((P, max(pl.ntails, 1)), np.float32)
        tvv = np.zeros((P, max(pl.ntails, 1)), np.float32)
        gidx_flat = []
        for spec in pl.chunks:
            # collect per-block index lists
            blk_idx = {}
            tail_feed = {}      # (quad, q) -> list of (idx, col_in_quad, v)
            for gg in spec.groups:
                qi = int(quad_of_g[gg])
                h = gg % QUAD
                for qq in range(NBUCKET):
                    eidx, erloc, ev = seg(gg, qq)
                    nf = int(nfull[gg][qq])
                    take = min(len(eidx), nf * P)
                    for bi in range(nf):
                        s0, s1 = bi * P, min((bi + 1) * P, take)
                        blk_idx.setdefault(("full", gg, qq), []).append(
                            (eidx[s0:s1], erloc[s0:s1], ev[s0:s1]))
                    rest = slice(take, len(eidx))
                    tail_feed.setdefault((qi, qq), []).append(
                        (eidx[rest], erloc[rest] + h * P, ev[rest]))
            # fill blocks in slot order
            for b in spec.blocks:
                if b.kind == "full":
                    lst = blk_idx.get(("full", b.g, b.q), [])
                    ei, er, ev = lst.pop(0) if lst else (
                        np.zeros(0, np.int64), np.zeros(0, np.int64),
                        np.zeros(0, np.float32))
                    b._fill = (ei, er, ev)
                else:
                    feeds = tail_feed.get((b.quad, b.q), [])
                    if feeds:
                        ei = np.concatenate([f[0] for f in feeds])
                        er = np.concatenate([f[1] for f in feeds])
                        ev = np.concatenate([f[2] for f in feeds])
                    else:
                        ei = np.zeros(0, np.int64)
                        er = np.zeros(0, np.int64)
                        ev = np.zeros(0, np.float32)
                    # this quad's tail blocks consume sequential slices
                    nprev = sum(1 for b2 in spec.blocks
                                if b2.kind == "tail" and b2.quad == b.quad
                                and b2.q == b.q and b2.pos[1] < b.pos[1])
                    s0, s1 = nprev * P, min((nprev + 1) * P, len(ei))
                    b._fill = (ei[s0:s1], er[s0:s1], ev[s0:s1])
            # write mb + idx per block (in slot order per q)
            for qq in range(NBUCKET):
                stream = [b for b in spec.blocks if b.q == qq]
                stream.sort(key=lambda b: b.pos[1])
                sidx = []
                for b in stream:
                    ei, er, ev = b._fill
                    n = len(ei)
                    full_slots = np.full(P, -1, np.int64)
                    full_slots[:n] = ei
                    # non-trailing padding: spread indices
                    pad = full_slots < 0
                    if pad.any():
                        full_slots[pad] = rng.integers(
                            0, bucket_rows, size=int(pad.sum()))
                    sidx.append(full_slots)
                    if b.kind == "full":
                        mb = np.zeros((P, b.mb_w), np.float32)
                        if n:
                            mb[np.arange(n), er] = ev
                        mbh[:, b.mb_off:b.mb_off + b.mb_w] = mb
                    else:
                        # tail: (dest-col, value) per slot; vv=0 kills pads
                        if n:
                            trl[:n, b.mb_off] = er.astype(np.float32)
                            tvv[:n, b.mb_off] = ev
                flat = (np.concatenate(sidx) if sidx
                        else np.zeros(0, np.int64))
                nidx = len(flat)
                assert nidx == spec.nq[qq] * P
                if nidx:
                    w = flat.reshape(nidx // 16, 16).T.astype(np.int16)
                    gidx_flat.append(np.tile(w, (8, 1)))
        gidx = (np.concatenate(gidx_flat, axis=1) if gidx_flat
                else np.zeros((P, 0), np.int16))
        assert gidx.shape == (P, pl.gidx_cols), (gidx.shape, pl.gidx_cols)
        pl.gidx.append(np.ascontiguousarray(gidx))
        pl.mbh.append(np.ascontiguousarray(
            mbh.astype(ml_dtypes.bfloat16)))
        pl.trl.append(np.ascontiguousarray(trl))
        pl.tvv.append(np.ascontiguousarray(
            tvv.astype(ml_dtypes.bfloat16).astype(np.float32)))
    return pl


# --------------------------------------------------------------------------
# device kernel build
# --------------------------------------------------------------------------

def build_nc(pl, stop_after=None, timing_variant=False, repeat=1,
             gd_bufs=2, mb_bufs=2, pg_bufs=3, x2_bufs=2, ag_after_chunk=4):
    ncores, local_pad, groups = pl.ncores, pl.local_pad, pl.groups
    BR, trows, CR = pl.bucket_rows, pl.trows, pl.chunk_rows
    stops = {"lin": 0, "ag": 1, "gatheronly": 2, "mbuild": 2.2, "mm": 2.5,
             "p2a": 2.8, "p2b": 2.9, "phase2": 3}
    level = stops.get(stop_after, 99)
    nquads = len(pl.quads)

    nc = bacc.Bacc("TRN2", target_bir_lowering=False, debug=False,
                   num_devices=ncores, enable_asserts=False,
                   num_swdge_queues=4)

    # inputs
    x2 = nc.dram_tensor("x2", [P, 2 * local_pad], BF16, kind="ExternalInput")
    w2 = nc.dram_tensor("w2", [P, 64], BF16, kind="ExternalInput")
    bias2 = nc.dram_tensor("bias2", [P, P], BF16, kind="ExternalInput")
    wbt = nc.dram_tensor("wbt", [64, 64], F32, kind="ExternalInput")
    acol = nc.dram_tensor("acol", [P, 1], F32, kind="ExternalInput")
    bbcol = nc.dram_tensor("bbcol", [P, 1], F32, kind="ExternalInput")
    gidx_d = nc.dram_tensor("gidx", [P, pl.gidx_cols], I16, kind="ExternalInput")
    mbh_d = nc.dram_tensor("mbh", [P, pl.mbh_cols], BF16, kind="ExternalInput")
    ntails = max(pl.ntails, 1)
    trl_d = nc.dram_tensor("trl", [P, ntails], F32, kind="ExternalInput")
    tvv_d = nc.dram_tensor("tvv", [P, ntails], F32, kind="ExternalInput")
    iot_d = nc.dram_tensor("iot", [P, QUAD * P], F32, kind="ExternalInput")

    scores_d = nc.dram_tensor("scores", [P, 2 * groups], F32,
                              kind="ExternalOutput")

    # internal DRAM (double-buffered across pipeline phases)
    xt_c = [nc.dram_tensor(f"xt_c{b}", [groups, P, P], BF16)
            for b in range(2)]
    if timing_variant:
        xt_all = [[nc.dram_tensor(f"xt_fake{b}_{k}", [BR, P], BF16,
                                  kind="ExternalInput")
                   for k in range(NBUCKET)] for b in range(2)]
        ar_in = [nc.dram_tensor(f"ar_in{b}", [64, 1], F32) for b in range(2)]
        ar_out = [nc.dram_tensor(f"ar_out{b}", [64, 1], F32)
                  for b in range(2)]
    else:
        xt_all = [[nc.dram_tensor(f"xt_all{b}_{k}", [BR, P], BF16,
                                  addr_space="Shared")
                   for k in range(NBUCKET)] for b in range(2)]
        ar_in = [nc.dram_tensor(f"ar_in{b}", [64, 1], F32) for b in range(2)]
        ar_out = [nc.dram_tensor(f"ar_out{b}", [64, 1], F32,
                                 addr_space="Shared") for b in range(2)]

    rg = [list(range(ncores))]
    inv_n = 1.0 / float(pl.local_n * ncores)

    with tile.TileContext(nc) as tc:
        nc.gpsimd.load_library(mlp)
        with (
            tc.tile_pool(name="const", bufs=1) as cpool,
            tc.tile_pool(name="big", bufs=1) as bigpool,
            tc.tile_pool(name="lin", bufs=x2_bufs) as lpool,
            tc.tile_pool(name="xrow", bufs=2) as xpool,
            tc.tile_pool(name="lpsum", bufs=2, space="PSUM") as lpsum,
            tc.tile_pool(name="gath", bufs=gd_bufs) as gpool,
            tc.tile_pool(name="mbst", bufs=mb_bufs) as mspool,
            tc.tile_pool(name="mbt", bufs=4) as mbpool,
            tc.tile_pool(name="gpsum", bufs=pg_bufs, space="PSUM") as gpsum,
            tc.tile_pool(name="ro", bufs=2) as ro,
            tc.tile_pool(name="rpsum", bufs=1, space="PSUM") as rpsum,
        ):
            w2_sb = cpool.tile([P, 64], BF16)
            nc.sync.dma_start(w2_sb[:], w2[:])
            bias2_sb = cpool.tile([P, P], BF16)
            nc.sync.dma_start(bias2_sb[:], bias2[:])
            wbt_sb = cpool.tile([64, 64], F32)
            nc.sync.dma_start(wbt_sb[:], wbt[:])
            a_sb = cpool.tile([P, 1], F32)
            nc.sync.dma_start(a_sb[:], acol[:])
            bb_sb = cpool.tile([P, 1], F32)
            nc.sync.dma_start(bb_sb[:], bbcol[:])
            idx_sb = bigpool.tile([P, pl.gidx_cols], I16)
            nc.sync.dma_start(idx_sb[:], gidx_d[:])
            trl_sb = cpool.tile([P, ntails], F32)
            nc.sync.dma_start(trl_sb[:], trl_d[:])
            tvv_sb = cpool.tile([P, ntails], F32)
            nc.sync.dma_start(tvv_sb[:], tvv_d[:])
            iot_sb = cpool.tile([P, QUAD * P], F32)
            nc.sync.dma_start(iot_sb[:], iot_d[:])

            HT = []
            acc = []
            for b in range(2):
                ht_t = bigpool.tile([P, local_pad], BF16, tag=f"HT{b}")
                HT.append(ht_t)
                acc_t = bigpool.tile([P, groups], F32, tag=f"acc{b}")
                acc.append(acc_t)

            # one-time memset of gather-dest pools so untriggered trailing
            # slots never contain non-finite garbage (0 * NaN = NaN in PE)
            nqmax = max(max(spec.nq) for spec in pl.chunks)
            gd_init = []
            for bi in range(gd_bufs):
                for qq in range(NBUCKET):
                    t = gpool.tile([P, nqmax, P], BF16, tag=f"gd{qq}")
                    nc.vector.memset(t[:], 0.0)
                    gd_init.append(t)

            def phase1(buf):
                """x2 -> xt_c[buf], chunked loads + batched stores."""
                nxch = (groups + GCHUNK - 1) // GCHUNK
                for ch in range(nxch):
                    g0 = ch * GCHUNK
                    g1 = min(g0 + GCHUNK, groups)
                    ng = g1 - g0
                    xin = lpool.tile([P, 2 * GCHUNK * P], BF16, tag="xin")
                    nc.sync.dma_start(xin[:, 0:ng * P],
                                      x2[:, g0 * P:g1 * P])
                    nc.sync.dma_start(
                        xin[:, GCHUNK * P:GCHUNK * P + ng * P],
                        x2[:, local_pad + g0 * P:local_pad + g1 * P])
                    xr = xpool.tile([P, GCHUNK * P], BF16, tag="xr")
                    for t in range(ng):
                        lp = xin[:, t * P:(t + 1) * P]
                        ln = xin[:, GCHUNK * P + t * P:GCHUNK * P + (t + 1) * P]
                        pt = lpsum.tile([P, P], F32, tag="pt")
                        nc.tensor.matmul(pt[:, 0:64], lhsT=lp, rhs=w2_sb[:],
                                         start=True, stop=True)
                        nc.tensor.matmul(pt[:, 64:128], lhsT=ln, rhs=w2_sb[:],
                                         start=True, stop=True)
                        nc.vector.scalar_tensor_tensor(
                            xr[:, t * P:(t + 1) * P], pt[:], 1.0, bias2_sb[:],
                            mybir.AluOpType.mult, mybir.AluOpType.add)
                    # batched store: DRAM [t, p, c] <- SBUF [p, (t, c)]
                    out_ap = xt_c[buf][g0:g1, :, :].rearrange("t p c -> p t c")
                    nc.sync.dma_start(out_ap, xr[:, 0:ng * P])

            def allgather(buf):
                if timing_variant:
                    return
                for k in range(NBUCKET):
                    src = xt_c[buf].ap().flatten_outer_dims()
                    nc.gpsimd.collective_compute(
                        "AllGather", mybir.AluOpType.bypass,
                        replica_groups=rg,
                        ins=[src[k * CR:(k + 1) * CR, :].opt()],
                        outs=[xt_all[buf][k].ap().opt()],
                    )

            def gather_chunk(buf, spec):
                gds = {}
                for qq in range(NBUCKET):
                    nq = spec.nq[qq]
                    if nq == 0:
                        continue
                    gd = gpool.tile([P, nqmax, P], BF16, tag=f"gd{qq}")
                    nidx = nq * P
                    qo = spec.q_off[qq]
                    nc.gpsimd.dma_gather(
                        gd[:, 0:nq, :], xt_all[buf][qq][:, :],
                        idx_sb[:, spec.idx_off + qo:
                               spec.idx_off + qo + nidx // 16],
                        nidx, nidx, P,
                        single_packet=(nidx <= 1024),
                        queue_num=qq,
                    )
                    gds[qq] = gd
                return gds

            def mm_chunk(buf, spec, gds, mb_sb):
                pgs = {}
                for qi in spec.quads:
                    pgs[qi] = gpsum.tile([P, QUAD * P], F32, tag="pg",
                                         name="pg")
                for b in spec.mm_order:
                    if level < 2.4:
                        break
                    qq, pos = b.pos
                    lhs = gds[qq][:, pos, :]
                    pg = pgs[b.quad]
                    if b.kind == "full":
                        rhs = mb_sb[:, b.mb_off - spec.mb_off0:
                                    b.mb_off - spec.mb_off0 + P]
                        h = b.g % QUAD
                        out = pg[:, h * P:(h + 1) * P]
                    else:
                        mbt = mbpool.tile([P, QUAD * P], BF16, tag="mbt")
                        nc.vector.tensor_scalar(
                            mbt[:, 0:b.mb_w], iot_sb[:, 0:b.mb_w],
                            trl_sb[:, b.mb_off:b.mb_off + 1],
                            tvv_sb[:, b.mb_off:b.mb_off + 1],
                            mybir.AluOpType.is_equal,
                            mybir.AluOpType.mult)
                        rhs = mbt[:, 0:b.mb_w]
                        out = pg[:, 0:b.mb_w]
                    nc.tensor.matmul(out, lhsT=lhs, rhs=rhs,
                                     start=b.start, stop=b.stop)
                if level < 2.7:
                    return
                for qi in spec.quads:
                    pg = pgs[qi]
                    for h, gg in enumerate(pl.quads[qi]):
                        nc.scalar.activation(
                            HT[buf][:, gg * P:(gg + 1) * P],
                            pg[:, h * P:(h + 1) * P],
                            mybir.ActivationFunctionType.Prelu,
                            alpha=a_sb[:, 0:1],
                            accum_out=acc[buf][:, gg:gg + 1])

            def phase2(buf, nxtbuf, do_ag_mid):
                """gathers+mm for rep buf; AG for nxt rep emitted mid-way."""
                for ci, spec in enumerate(pl.chunks):
                    if ci == ag_after_chunk and do_ag_mid:
                        allgather(nxtbuf)
                    mb_sb = None
                    if level >= 2.1:
                        mb_sb = mspool.tile([P, max(s.mb_len for s in pl.chunks)],
                                            BF16, tag="mbs")
                        nc.scalar.dma_start(
                            mb_sb[:, 0:spec.mb_len],
                            mbh_d[:, spec.mb_off0:spec.mb_off0 + spec.mb_len])
                    gds = gather_chunk(buf, spec)
                    if level >= 2.4 and mb_sb is not None:
                        mm_chunk(buf, spec, gds, mb_sb)
                if do_ag_mid and ag_after_chunk >= len(pl.chunks):
                    allgather(nxtbuf)

            def allreduce(buf):
                msum = ro.tile([P, 1], F32, tag="msum")
                nc.vector.reduce_sum(msum[:], acc[buf][:],
                                     axis=mybir.AxisListType.X)
                nc.sync.dma_start(ar_in[buf][:], msum[0:64, :])
                if timing_variant:
                    arb = ro.tile([64, 1], F32, tag="arb")
                    nc.sync.dma_start(arb[:], ar_in[buf][:])
                    nc.sync.dma_start(ar_out[buf][:], arb[:])
                else:
                    nc.gpsimd.collective_compute(
                        "AllReduce", mybir.AluOpType.add, replica_groups=rg,
                        ins=[ar_in[buf].ap().opt()],
                        outs=[ar_out[buf].ap().opt()],
                    )

            def phase3(buf):
                ssum = ro.tile([64, 1], F32, tag="ssum")
                nc.sync.dma_start(ssum[:], ar_out[buf][:])
                sig = ro.tile([64, 1], F32, tag="sig")
                nc.scalar.activation(sig[:], ssum[:],
                                     mybir.ActivationFunctionType.Sigmoid,
                                     scale=inv_n)
                zp = rpsum.tile([64, 1], F32, tag="zp")
                nc.tensor.matmul(zp[:], lhsT=wbt_sb[:], rhs=sig[:],
                                 start=True, stop=True)
                z2 = ro.tile([P, 2], BF16, tag="z2")
                nc.vector.memset(z2[:], 0.0)
                nc.scalar.copy(z2[0:64, 0:1], zp[:])
                nc.scalar.copy(z2[64:128, 1:2], zp[:])
                sp = rpsum.tile([P, 2 * groups], F32, tag="sp")
                for t in range(groups):
                    nc.tensor.matmul(sp[:, 2 * t:2 * t + 2],
                                     lhsT=HT[buf][:, t * P:(t + 1) * P],
                                     rhs=z2[:], start=True, stop=True)
                scr = ro.tile([P, 2 * groups], F32, tag="scr")
                nc.vector.tensor_scalar_add(scr[:], sp[:], bb_sb[:, 0:1])
                nc.sync.dma_start(scores_d[:], scr[:])

            # ---------------- pipelined schedule ----------------
            if level < 1:
                for i in range(repeat):
                    phase1(i % 2)
                _early_out(nc, tc, scores_d, groups)
            elif level < 2:
                for i in range(repeat):
                    phase1(i % 2)
                    allgather(i % 2)
                _early_out(nc, tc, scores_d, groups)
            else:
                phase1(0)
                allgather(0)
                for i in range(repeat):
                    cur, nxt = i % 2, (i + 1) % 2
                    if i + 1 < repeat:
                        phase1(nxt)
                    phase2(cur, nxt, do_ag_mid=(i + 1 < repeat))
                    if level >= 4:
                        allreduce(cur)
                        phase3(cur)
                if level < 4:
                    _early_out(nc, tc, scores_d, groups)

    nc.compile()
    return nc


class _EarlyStop(Exception):
    pass


def _early_out(nc, tc, scores_d, groups):
    with tc.tile_pool(name="eo", bufs=1) as eo:
        scr = eo.tile([P, 2 * groups], F32)
        nc.vector.memset(scr[:], 0.0)
        nc.sync.dma_start(scores_d[:], scr[:])


# --------------------------------------------------------------------------
# host glue
# --------------------------------------------------------------------------

def _make_in_maps(pl, inputs):
    ncores = pl.ncores
    pos, neg = inputs["pos"], inputs["neg"]
    local_n, local_pad = pl.local_n, pl.local_pad
    a_val = np.float32(np.asarray(inputs["prelu_a"]).reshape(-1)[0])
    bb_val = np.float32(np.asarray(inputs["b_bil"]).reshape(-1)[0])
    w2 = np.ascontiguousarray(
        np.asarray(inputs["W_gcn"]).T.astype(ml_dtypes.bfloat16))
    wbt = np.ascontiguousarray(np.asarray(inputs["W_bil"]).T.astype(np.float32))
    bgv = np.asarray(inputs["b_gcn"]).reshape(-1).astype(np.float32)
    bias2 = np.tile(np.concatenate([bgv, bgv])[None, :], (P, 1)).astype(
        ml_dtypes.bfloat16)

    posT = np.asarray(pos[0]).T.astype(ml_dtypes.bfloat16)   # [128, N]
    negT = np.asarray(neg[0]).T.astype(ml_dtypes.bfloat16)

    in_maps = []
    for c in range(ncores):
        sl = slice(c * local_n, (c + 1) * local_n)
        x2 = np.zeros((P, 2 * local_pad), ml_dtypes.bfloat16)
        x2[:, :local_n] = posT[:, sl]
        x2[:, local_pad:local_pad + local_n] = negT[:, sl]
        in_maps.append({
            "x2": x2,
            "mbh": pl.mbh[c],
            "trl": pl.trl[c],
            "tvv": pl.tvv[c],
            "iot": np.tile(np.arange(QUAD * P, dtype=np.float32)[None, :],
                           (P, 1)),
            "w2": w2,
            "bias2": bias2,
            "wbt": wbt,
            "acol": np.full((P, 1), a_val, np.float32),
            "bbcol": np.full((P, 1), bb_val, np.float32),
            "gidx": pl.gidx[c],
        })
    return in_maps


def _assemble(pl, results, n_total):
    ncores, local_n, local_pad = pl.ncores, pl.local_n, pl.local_pad
    logits = np.zeros((1, 2 * n_total), np.float32)
    for c in range(ncores):
        arr = results[c]["scores"]            # [P, 2*groups]
        posv = arr[:, 0::2].T.reshape(local_pad)[:local_n]
        negv = arr[:, 1::2].T.reshape(local_pad)[:local_n]
        logits[0, c * local_n:(c + 1) * local_n] = posv
        logits[0, n_total + c * local_n:n_total + (c + 1) * local_n] = negv
    return logits


def _run(pos, neg, edge_rows, edge_cols, edge_vals,
         W_gcn, b_gcn, prelu_a, W_bil, b_bil, ncores=8, **run_kwargs):
    n_nodes = pos.shape[1]
    f_dim = pos.shape[2]
    assert f_dim == P

    pl = make_plan(n_nodes, ncores, edge_rows, edge_cols, edge_vals)

    key = (n_nodes, ncores, pl.b_total, pl.gidx_cols, pl.mbh_cols)
    if key in _NC_CACHE:
        nc = _NC_CACHE[key]
    else:
        nc = build_nc(pl)
        _NC_CACHE.clear()
        _NC_CACHE[key] = nc

    in_maps = _make_in_maps(pl, {
        "pos": pos, "neg": neg, "W_gcn": W_gcn, "b_gcn": b_gcn,
        "prelu_a": prelu_a, "W_bil": W_bil, "b_bil": b_bil,
    })

    res = run_bass_kernel_spmd(nc, in_maps, core_ids=list(range(ncores)),
                               **run_kwargs)

    logits = _assemble(pl, res.results, n_nodes)
    return logits, res


def kernel(pos, neg, edge_rows, edge_cols, edge_vals,
           W_gcn, b_gcn, prelu_a, W_bil, b_bil):
    logits, _ = _run(pos, neg, edge_rows, edge_cols, edge_vals,
                     W_gcn, b_gcn, prelu_a, W_bil, b_bil)
    return logits


# revision 32
# speedup vs baseline: 1.2677x; 1.2677x over previous
"""DGI (Deep Graph Infomax) kernel for 8 Trainium2 NeuronCores.

Strategy (row-wise graph partitioning + cross-iteration software pipeline):
  - nodes split 12500/core (padded to 12544 = 98*128 rows); each core owns
    the incoming edges of its node block.
  - phase 1: each core computes its shard of xtheta = x @ W^T + b for both
    graphs in row layout, rows stored as [node, pos_h(64) | neg_h(64)] bf16
    (256B rows). Stores are batched: one strided DMA per 8-group slab
    (DRAM AP transposed "t p c -> p t c") instead of 98 x 32KB stores.
  - the AllGather is split into 4 row-chunk collectives (3136 rows/chunk):
    AG_k concatenates chunk k of every rank -> xt_all_k [25088, 128].
    Gather buckets are keyed by SOURCE CHUNK (not rank pair), so each
    bucket's table is one contiguous xt_all_k and bucket-k gathers depend
    only on AG_k. Same block padding as rank-pair bucketing.
  - per-edge gather of 256B source rows via gpsimd dma_gather on 4 SWDGE
    queues (queue q <-> bucket q). Padding slots use spread indices (all-
    row-0 padding serializes on one HBM port); trailing padding of each
    call is -1 (ucode trims negative tail indices before descgen).
  - segment-sum via matmul: full 128-edge blocks target a 128-col slice of
    a [128, 512] PSUM tile (quad of 4 dest groups); the tails of the 4
    groups are merged into shared blocks with a 512-wide host-built M
    (arbitrary sparse rhs, not just one-hot), cutting gather slots ~10%.
  - PReLU + mean-readout row-sum fused into one Activation op per group.
  - cross-rep software pipeline: xt_c/xt_all/HT/acc/ar are double-buffered
    and emission is skewed [ph1(i+1); gathers(i) part A; AG(i+1) x4;
    gathers(i) part B; AR(i); ph3(i)] so collectives and phase 1/3 hide
    under the gather descriptor-generation wall (the serial resource).
"""

import sys

try:
    import concourse.bacc as bacc
except ImportError:  # pragma: no cover
    sys.path.insert(0, "/opt/trn_rl_repo")
    import concourse.bacc as bacc

import numpy as np
import ml_dtypes

import concourse.bass as bass
import concourse.mybir as mybir
import concourse.tile as tile
from concourse.library_config import mlp
from concourse.bass_utils import run_bass_kernel_spmd

P = 128
QUAD = 4            # dest groups merged per PSUM tile
GCHUNK = 4          # dest groups per gather/matmul chunk (1 quad)
NBUCKET = 4         # source chunks (= SWDGE queues)
BF16 = mybir.dt.bfloat16
F32 = mybir.dt.float32
I16 = mybir.dt.int16

_NC_CACHE = {}


# --------------------------------------------------------------------------
# host-side planning
# --------------------------------------------------------------------------

class Plan:
    pass


class Block:
    """One 128-slot gather/matmul block.

    kind 'full': 128 edges of one dest group  -> 128-wide mb slice
    kind 'tail': merged tail edges of a quad  -> (128*quad_len)-wide mb
    """
    __slots__ = ("kind", "g", "quad", "q", "pos", "mb_off", "mb_w",
                 "start", "stop", "slices", "_fill")

    def __init__(self, kind, g, quad, q):
        self.kind = kind
        self.g = g          # dest group (fulls) or first group of quad
        self.quad = quad
        self.q = q


def make_plan(n_nodes, ncores, edge_rows, edge_cols, edge_vals,
              g_chunk=GCHUNK):
    pl = Plan()
    assert g_chunk % QUAD == 0
    pl.g_chunk = g_chunk
    local_n = n_nodes // ncores
    assert local_n * ncores == n_nodes
    local_pad = ((local_n + P - 1) // P) * P
    groups = local_pad // P
    chunk_rows = local_pad // NBUCKET          # 3136
    assert chunk_rows * NBUCKET == local_pad
    bucket_rows = ncores * chunk_rows          # 25088 <= 32767
    assert bucket_rows <= 32767

    pl.ncores, pl.local_n, pl.local_pad = ncores, local_n, local_pad
    pl.groups, pl.chunk_rows, pl.bucket_rows = groups, chunk_rows, bucket_rows
    pl.trows = ncores * local_pad

    r = np.asarray(edge_rows).astype(np.int64)
    c = np.asarray(edge_cols).astype(np.int64)
    v = np.asarray(edge_vals).astype(np.float32)

    core = r // local_n
    lr = r % local_n
    g = lr // P
    crank = c // local_n
    cloc = c % local_n
    q = cloc // chunk_rows                      # source-chunk bucket
    idx16 = crank * chunk_rows + (cloc - q * chunk_rows)

    # quad structure over dest groups
    quads = [list(range(k, min(k + QUAD, groups))) for k in range(0, groups, QUAD)]
    pl.quads = quads
    quad_of_g = np.zeros(groups, np.int64)
    for qi, gs in enumerate(quads):
        for gg in gs:
            quad_of_g[gg] = qi

    # per (core, g, q) edge counts -> shared (max-over-core) block layout
    key = (core * groups + g) * NBUCKET + q
    counts = np.bincount(key, minlength=ncores * groups * NBUCKET).reshape(
        ncores, groups, NBUCKET)
    cmax = counts.max(axis=0)                  # [groups, NBUCKET]
    nfull = cmax // P                          # full blocks per (g, q)
    tail = cmax - nfull * P                    # tail edges per (g, q)
    # every group needs >=1 block writer per 128-col psum slice; if a group
    # has no full block in any q, promote its largest tail to a full block
    for gg in range(groups):
        if nfull[gg].sum() == 0:
            qq = int(np.argmax(tail[gg]))
            nfull[gg][qq] = 1
            tail[gg][qq] = 0
    # merged tail blocks per (quad, q)
    tailsum = np.zeros((len(quads), NBUCKET), np.int64)
    for qi, gs in enumerate(quads):
        for qq in range(NBUCKET):
            tailsum[qi][qq] = sum(int(tail[gg][qq]) for gg in gs)
    ntail = (tailsum + P - 1) // P

    # chunk structure (g_chunk dest groups per chunk)
    chunk_gs = [list(range(k, min(k + g_chunk, groups)))
                for k in range(0, groups, g_chunk)]
    pl.chunks = []
    mb_off = 0          # bf16 cols into mbh (full blocks only)
    tail_off = 0        # tail-block counter (trl/tvv columns)
    idx_off = 0         # int16 cols into gidx
    jslot = 0           # global block slot counter
    for gs in chunk_gs:
        spec = Plan()
        spec.groups = gs
        spec.quads = sorted({int(quad_of_g[gg]) for gg in gs})
        spec.idx_off = idx_off
        spec.q_off = []
        spec.nq = []
        spec.blocks = []            # flat list of Block in gd-slot order
        for qq in range(NBUCKET):
            spec.q_off.append(idx_off - spec.idx_off)
            nq = 0
            for gg in gs:
                for _ in range(int(nfull[gg][qq])):
                    b = Block("full", gg, int(quad_of_g[gg]), qq)
                    b.pos = (qq, nq)
                    spec.blocks.append(b)
                    nq += 1
            for qi in spec.quads:
                for _ in range(int(ntail[qi][qq])):
                    b = Block("tail", pl.quads[qi][0], qi, qq)
                    b.pos = (qq, nq)
                    spec.blocks.append(b)
                    nq += 1
            spec.nq.append(nq)
            idx_off += nq * 8       # nq*128 idx -> /16 cols
        spec.idx_len = idx_off - spec.idx_off
        # mb layout: fulls host-streamed (128 cols each); tails DVE-built
        # from (trl, tvv) columns (one per tail block)
        spec.mb_off0 = mb_off
        for b in spec.blocks:
            b.mb_w = P if b.kind == "full" else P * len(pl.quads[b.quad])
            if b.kind == "full":
                b.mb_off = mb_off
                mb_off += P
            else:
                b.mb_off = tail_off
                tail_off += 1
        spec.mb_len = mb_off - spec.mb_off0
        jslot += sum(spec.nq)
        pl.chunks.append(spec)
    pl.b_total = jslot
    pl.gidx_cols = idx_off
    pl.mbh_cols = mb_off
    pl.ntails = tail_off

    # ---- matmul emission order + start/stop flags ----
    # PSUM accumulation groups are 2KB-bank granular: start=True marks the
    # whole bank pending-zero (first write per address overwrites, later
    # writes accumulate). So per quad tile: start on the FIRST matmul into
    # the tile, stop on the LAST, regardless of column slice.
    for spec in pl.chunks:
        fulls = [b for b in spec.blocks if b.kind == "full"]
        tails = [b for b in spec.blocks if b.kind == "tail"]
        spec.mm_order = fulls + tails
        first_q = {}
        last_q = {}
        for b in spec.mm_order:
            if b.quad not in first_q:
                first_q[b.quad] = b
            last_q[b.quad] = b
        for b in spec.mm_order:
            b.start = first_q[b.quad] is b
            b.stop = last_q[b.quad] is b

    # ---- per-core edge data ----
    order = np.lexsort((idx16, q, g, core))
    so_core, so_g, so_q = core[order], g[order], q[order]
    so_idx, so_lr, so_v = idx16[order], lr[order], v[order]
    seg_key = (so_core * groups + so_g) * NBUCKET + so_q
    seg_counts = np.bincount(seg_key, minlength=ncores * groups * NBUCKET)
    seg_starts = np.concatenate([[0], np.cumsum(seg_counts)])

    pl.gidx = []
    pl.mbh = []
    pl.trl = []
    pl.tvv = []
    rng = np.random.default_rng(1234)
    for cc in range(ncores):
        # per (g, q): this core's edges (idx, rloc, v), rloc relative to
        # the group's 128 rows
        def seg(gg, qq):
            sk = (cc * groups + gg) * NBUCKET + qq
            s0, s1 = seg_starts[sk], seg_starts[sk + 1]
            return (so_idx[s0:s1], (so_lr[s0:s1] - gg * P), so_v[s0:s1])

        # distribute edges to blocks: per (g,q) first nfull*P edges fill
        # the full blocks; the remainder goes to the quad's tail blocks.
        mbh = np.zeros((P, pl.mbh_cols), np.float32)
        trl = np.zeros((P, max(pl.ntails, 1)), np.float32)
        tvv = np.zeros((P, max(pl.ntails, 1)), np.float32)
        gidx_flat = []
        for spec in pl.chunks:
            # collect per-block index lists
            blk_idx = {}
            tail_feed = {}      # (quad, q) -> list of (idx, col_in_quad, v)
            for gg in spec.groups:
                qi = int(quad_of_g[gg])
                h = gg % QUAD
                for qq in range(NBUCKET):
                    eidx, erloc, ev = seg(gg, qq)
                    nf = int(nfull[gg][qq])
                    take = min(len(eidx), nf * P)
                    for bi in range(nf):
                        s0, s1 = bi * P, min((bi + 1) * P, take)
                        blk_idx.setdefault(("full", gg, qq), []).append(
                            (eidx[s0:s1], erloc[s0:s1], ev[s0:s1]))
                    rest = slice(take, len(eidx))
                    tail_feed.setdefault((qi, qq), []).append(
                        (eidx[rest], erloc[rest] + h * P, ev[rest]))
            # fill blocks in slot order
            for b in spec.blocks:
                if b.kind == "full":
                    lst = blk_idx.get(("full", b.g, b.q), [])
                    ei, er, ev = lst.pop(0) if lst else (
                        np.zeros(0, np.int64), np.zeros(0, np.int64),
                        np.zeros(0, np.float32))
                    b._fill = (ei, er, ev)
                else:
                    feeds = tail_feed.get((b.quad, b.q), [])
                    if feeds:
                        ei = np.concatenate([f[0] for f in feeds])
                        er = np.concatenate([f[1] for f in feeds])
                        ev = np.concatenate([f[2] for f in feeds])
                    else:
                        ei = np.zeros(0, np.int64)
                        er = np.zeros(0, np.int64)
                        ev = np.zeros(0, np.float32)
                    # this quad's tail blocks consume sequential slices
                    nprev = sum(1 for b2 in spec.blocks
                                if b2.kind == "tail" and b2.quad == b.quad
                                and b2.q == b.q and b2.pos[1] < b.pos[1])
                    s0, s1 = nprev * P, min((nprev + 1) * P, len(ei))
                    b._fill = (ei[s0:s1], er[s0:s1], ev[s0:s1])
            # write mb + idx per block (in slot order per q)
            for qq in range(NBUCKET):
                stream = [b for b in spec.blocks if b.q == qq]
                stream.sort(key=lambda b: b.pos[1])
                sidx = []
                for b in stream:
                    ei, er, ev = b._fill
                    n = len(ei)
                    full_slots = np.full(P, -1, np.int64)
                    full_slots[:n] = ei
                    # non-trailing padding: spread indices
                    pad = full_slots < 0
                    if pad.any():
                        full_slots[pad] = rng.integers(
                            0, bucket_rows, size=int(pad.sum()))
                    sidx.append(full_slots)
                    if b.kind == "full":
                        mb = np.zeros((P, b.mb_w), np.float32)
                        if n:
                            mb[np.arange(n), er] = ev
                        mbh[:, b.mb_off:b.mb_off + b.mb_w] = mb
                    else:
                        # tail: (dest-col, value) per slot; vv=0 kills pads
                        if n:
                            trl[:n, b.mb_off] = er.astype(np.float32)
                            tvv[:n, b.mb_off] = ev
                flat = (np.concatenate(sidx) if sidx
                        else np.zeros(0, np.int64))
                nidx = len(flat)
                assert nidx == spec.nq[qq] * P
                if nidx:
                    w = flat.reshape(nidx // 16, 16).T.astype(np.int16)
                    gidx_flat.append(np.tile(w, (8, 1)))
        gidx = (np.concatenate(gidx_flat, axis=1) if gidx_flat
                else np.zeros((P, 0), np.int16))
        assert gidx.shape == (P, pl.gidx_cols), (gidx.shape, pl.gidx_cols)
        pl.gidx.append(np.ascontiguousarray(gidx))
        pl.mbh.append(np.ascontiguousarray(
            mbh.astype(ml_dtypes.bfloat16)))
        pl.trl.append(np.ascontiguousarray(trl))
        pl.tvv.append(np.ascontiguousarray(
            tvv.astype(ml_dtypes.bfloat16).astype(np.float32)))
    return pl


# --------------------------------------------------------------------------
# device kernel build
# --------------------------------------------------------------------------

def build_nc(pl, stop_after=None, timing_variant=False, repeat=1,
             gd_bufs=3, mb_bufs=2, pg_bufs=3, x2_bufs=2, ag_after_chunk=8,
             fake_tails=False):
    ncores, local_pad, groups = pl.ncores, pl.local_pad, pl.groups
    BR, trows, CR = pl.bucket_rows, pl.trows, pl.chunk_rows
    stops = {"lin": 0, "ag": 1, "gatheronly": 2, "mbuild": 2.2, "mm": 2.5,
             "p2a": 2.8, "p2b": 2.9, "phase2": 3}
    level = stops.get(stop_after, 99)
    nquads = len(pl.quads)

    nc = bacc.Bacc("TRN2", target_bir_lowering=False, debug=False,
                   num_devices=ncores, enable_asserts=False,
                   num_swdge_queues=4)

    # inputs
    x2 = nc.dram_tensor("x2", [P, 2 * local_pad], BF16, kind="ExternalInput")
    w2 = nc.dram_tensor("w2", [P, 64], BF16, kind="ExternalInput")
    bias2 = nc.dram_tensor("bias2", [P, P], BF16, kind="ExternalInput")
    wbt = nc.dram_tensor("wbt", [64, 64], F32, kind="ExternalInput")
    acol = nc.dram_tensor("acol", [P, 1], F32, kind="ExternalInput")
    bbcol = nc.dram_tensor("bbcol", [P, 1], F32, kind="ExternalInput")
    gidx_d = nc.dram_tensor("gidx", [P, pl.gidx_cols], I16, kind="ExternalInput")
    mbh_d = nc.dram_tensor("mbh", [P, pl.mbh_cols], BF16, kind="ExternalInput")
    ntails = max(pl.ntails, 1)
    trl_d = nc.dram_tensor("trl", [P, ntails], F32, kind="ExternalInput")
    tvv_d = nc.dram_tensor("tvv", [P, ntails], F32, kind="ExternalInput")
    iot_d = nc.dram_tensor("iot", [P, QUAD * P], F32, kind="ExternalInput")

    scores_d = nc.dram_tensor("scores", [P, 2 * groups], F32,
                              kind="ExternalOutput")

    # internal DRAM (double-buffered across pipeline phases)
    xt_c = [nc.dram_tensor(f"xt_c{b}", [groups, P, P], BF16)
            for b in range(2)]
    if timing_variant:
        xt_all = [[nc.dram_tensor(f"xt_fake{b}_{k}", [BR, P], BF16,
                                  kind="ExternalInput")
                   for k in range(NBUCKET)] for b in range(2)]
        ar_in = [nc.dram_tensor(f"ar_in{b}", [64, 1], F32) for b in range(2)]
        ar_out = [nc.dram_tensor(f"ar_out{b}", [64, 1], F32)
                  for b in range(2)]
    else:
        xt_all = [[nc.dram_tensor(f"xt_all{b}_{k}", [BR, P], BF16,
                                  addr_space="Shared")
                   for k in range(NBUCKET)] for b in range(2)]
        ar_in = [nc.dram_tensor(f"ar_in{b}", [64, 1], F32) for b in range(2)]
        ar_out = [nc.dram_tensor(f"ar_out{b}", [64, 1], F32,
                                 addr_space="Shared") for b in range(2)]

    rg = [list(range(ncores))]
    inv_n = 1.0 / float(pl.local_n * ncores)

    with tile.TileContext(nc) as tc:
        nc.gpsimd.load_library(mlp)
        with (
            tc.tile_pool(name="const", bufs=1) as cpool,
            tc.tile_pool(name="big", bufs=1) as bigpool,
            tc.tile_pool(name="lin", bufs=x2_bufs) as lpool,
            tc.tile_pool(name="xrow", bufs=2) as xpool,
            tc.tile_pool(name="lpsum", bufs=2, space="PSUM") as lpsum,
            tc.tile_pool(name="gath", bufs=gd_bufs) as gpool,
            tc.tile_pool(name="mbst", bufs=mb_bufs) as mspool,
            tc.tile_pool(name="mbt", bufs=4) as mbpool,
            tc.tile_pool(name="gpsum", bufs=pg_bufs, space="PSUM") as gpsum,
            tc.tile_pool(name="ro", bufs=2) as ro,
            tc.tile_pool(name="rpsum", bufs=1, space="PSUM") as rpsum,
        ):
            w2_sb = cpool.tile([P, 64], BF16)
            nc.sync.dma_start(w2_sb[:], w2[:])
            bias2_sb = cpool.tile([P, P], BF16)
            nc.sync.dma_start(bias2_sb[:], bias2[:])
            wbt_sb = cpool.tile([64, 64], F32)
            nc.sync.dma_start(wbt_sb[:], wbt[:])
            a_sb = cpool.tile([P, 1], F32)
            nc.sync.dma_start(a_sb[:], acol[:])
            bb_sb = cpool.tile([P, 1], F32)
            nc.sync.dma_start(bb_sb[:], bbcol[:])
            idx_sb = bigpool.tile([P, pl.gidx_cols], I16)
            nc.sync.dma_start(idx_sb[:], gidx_d[:])
            trl_sb = cpool.tile([P, ntails], F32)
            nc.sync.dma_start(trl_sb[:], trl_d[:])
            tvv_sb = cpool.tile([P, ntails], F32)
            nc.sync.dma_start(tvv_sb[:], tvv_d[:])
            iot_sb = cpool.tile([P, QUAD * P], F32)
            nc.sync.dma_start(iot_sb[:], iot_d[:])

            HT = []
            acc = []
            for b in range(2):
                ht_t = bigpool.tile([P, local_pad], BF16, tag=f"HT{b}")
                HT.append(ht_t)
                acc_t = bigpool.tile([P, groups], F32, tag=f"acc{b}")
                acc.append(acc_t)

            # one-time memset of gather-dest pools so untriggered trailing
            # slots never contain non-finite garbage (0 * NaN = NaN in PE)
            nqmax = max(max(spec.nq) for spec in pl.chunks)
            gd_init = []
            for bi in range(gd_bufs):
                for qq in range(NBUCKET):
                    t = gpool.tile([P, nqmax, P], BF16, tag=f"gd{qq}")
                    nc.vector.memset(t[:], 0.0)
                    gd_init.append(t)

            def phase1(buf):
                """x2 -> xt_c[buf], chunked loads + batched stores."""
                nxch = (groups + GCHUNK - 1) // GCHUNK
                for ch in range(nxch):
                    g0 = ch * GCHUNK
                    g1 = min(g0 + GCHUNK, groups)
                    ng = g1 - g0
                    xin = lpool.tile([P, 2 * GCHUNK * P], BF16, tag="xin")
                    nc.sync.dma_start(xin[:, 0:ng * P],
                                      x2[:, g0 * P:g1 * P])
                    nc.sync.dma_start(
                        xin[:, GCHUNK * P:GCHUNK * P + ng * P],
                        x2[:, local_pad + g0 * P:local_pad + g1 * P])
                    xr = xpool.tile([P, GCHUNK * P], BF16, tag="xr")
                    for t in range(ng):
                        lp = xin[:, t * P:(t + 1) * P]
                        ln = xin[:, GCHUNK * P + t * P:GCHUNK * P + (t + 1) * P]
                        pt = lpsum.tile([P, P], F32, tag="pt")
                        nc.tensor.matmul(pt[:, 0:64], lhsT=lp, rhs=w2_sb[:],
                                         start=True, stop=True)
                        nc.tensor.matmul(pt[:, 64:128], lhsT=ln, rhs=w2_sb[:],
                                         start=True, stop=True)
                        nc.vector.scalar_tensor_tensor(
                            xr[:, t * P:(t + 1) * P], pt[:], 1.0, bias2_sb[:],
                            mybir.AluOpType.mult, mybir.AluOpType.add)
                    # batched store: DRAM [t, p, c] <- SBUF [p, (t, c)]
                    out_ap = xt_c[buf][g0:g1, :, :].rearrange("t p c -> p t c")
                    nc.sync.dma_start(out_ap, xr[:, 0:ng * P])

            def allgather(buf):
                if timing_variant:
                    return
                for k in range(NBUCKET):
                    src = xt_c[buf].ap().flatten_outer_dims()
                    nc.gpsimd.collective_compute(
                        "AllGather", mybir.AluOpType.bypass,
                        replica_groups=rg,
                        ins=[src[k * CR:(k + 1) * CR, :].opt()],
                        outs=[xt_all[buf][k].ap().opt()],
                    )

            def gather_chunk(buf, spec):
                gds = {}
                for qq in range(NBUCKET):
                    nq = spec.nq[qq]
                    if nq == 0:
                        continue
                    gd = gpool.tile([P, nqmax, P], BF16, tag=f"gd{qq}")
                    nidx = nq * P
                    qo = spec.q_off[qq]
                    nc.gpsimd.dma_gather(
                        gd[:, 0:nq, :], xt_all[buf][qq][:, :],
                        idx_sb[:, spec.idx_off + qo:
                               spec.idx_off + qo + nidx // 16],
                        nidx, nidx, P,
                        single_packet=(nidx <= 1024),
                        queue_num=qq,
                    )
                    gds[qq] = gd
                return gds

            def mm_chunk(buf, spec, gds, mb_sb):
                pgs = {}
                for qi in spec.quads:
                    pgs[qi] = gpsum.tile([P, QUAD * P], F32, tag="pg",
                                         name="pg")
                for b in spec.mm_order:
                    if level < 2.4:
                        break
                    qq, pos = b.pos
                    lhs = gds[qq][:, pos, :]
                    pg = pgs[b.quad]
                    if b.kind == "full":
                        rhs = mb_sb[:, b.mb_off - spec.mb_off0:
                                    b.mb_off - spec.mb_off0 + P]
                        h = b.g % QUAD
                        out = pg[:, h * P:(h + 1) * P]
                    elif fake_tails:
                        rhs = mb_sb[:, 0:b.mb_w]
                        out = pg[:, 0:b.mb_w]
                    else:
                        mbt = mbpool.tile([P, QUAD * P], BF16, tag="mbt")
                        nc.vector.tensor_scalar(
                            mbt[:, 0:b.mb_w], iot_sb[:, 0:b.mb_w],
                            trl_sb[:, b.mb_off:b.mb_off + 1],
                            tvv_sb[:, b.mb_off:b.mb_off + 1],
                            mybir.AluOpType.is_equal,
                            mybir.AluOpType.mult)
                        rhs = mbt[:, 0:b.mb_w]
                        out = pg[:, 0:b.mb_w]
                    nc.tensor.matmul(out, lhsT=lhs, rhs=rhs,
                                     start=b.start, stop=b.stop)
                if level < 2.7:
                    return
                for qi in spec.quads:
                    pg = pgs[qi]
                    for h, gg in enumerate(pl.quads[qi]):
                        nc.scalar.activation(
                            HT[buf][:, gg * P:(gg + 1) * P],
                            pg[:, h * P:(h + 1) * P],
                            mybir.ActivationFunctionType.Prelu,
                            alpha=a_sb[:, 0:1],
                            accum_out=acc[buf][:, gg:gg + 1])

            def phase2(buf, nxtbuf, do_ag_mid):
                """gathers+mm for rep buf; AG for nxt rep emitted mid-way."""
                for ci, spec in enumerate(pl.chunks):
                    if ci == ag_after_chunk and do_ag_mid:
                        allgather(nxtbuf)
                    mb_sb = None
                    if level >= 2.1:
                        mb_sb = mspool.tile([P, max(s.mb_len for s in pl.chunks)],
                                            BF16, tag="mbs")
                        nc.scalar.dma_start(
                            mb_sb[:, 0:spec.mb_len],
                            mbh_d[:, spec.mb_off0:spec.mb_off0 + spec.mb_len])
                    gds = gather_chunk(buf, spec)
                    if level >= 2.4 and mb_sb is not None:
                        mm_chunk(buf, spec, gds, mb_sb)
                if do_ag_mid and ag_after_chunk >= len(pl.chunks):
                    allgather(nxtbuf)

            def allreduce(buf):
                msum = ro.tile([P, 1], F32, tag="msum")
                nc.vector.reduce_sum(msum[:], acc[buf][:],
                                     axis=mybir.AxisListType.X)
                nc.sync.dma_start(ar_in[buf][:], msum[0:64, :])
                if timing_variant:
                    arb = ro.tile([64, 1], F32, tag="arb")
                    nc.sync.dma_start(arb[:], ar_in[buf][:])
                    nc.sync.dma_start(ar_out[buf][:], arb[:])
                else:
                    nc.gpsimd.collective_compute(
                        "AllReduce", mybir.AluOpType.add, replica_groups=rg,
                        ins=[ar_in[buf].ap().opt()],
                        outs=[ar_out[buf].ap().opt()],
                    )

            def phase3(buf):
                ssum = ro.tile([64, 1], F32, tag="ssum")
                nc.sync.dma_start(ssum[:], ar_out[buf][:])
                sig = ro.tile([64, 1], F32, tag="sig")
                nc.scalar.activation(sig[:], ssum[:],
                                     mybir.ActivationFunctionType.Sigmoid,
                                     scale=inv_n)
                zp = rpsum.tile([64, 1], F32, tag="zp")
                nc.tensor.matmul(zp[:], lhsT=wbt_sb[:], rhs=sig[:],
                                 start=True, stop=True)
                z2 = ro.tile([P, 2], BF16, tag="z2")
                nc.vector.memset(z2[:], 0.0)
                nc.scalar.copy(z2[0:64, 0:1], zp[:])
                nc.scalar.copy(z2[64:128, 1:2], zp[:])
                sp = rpsum.tile([P, 2 * groups], F32, tag="sp")
                for t in range(groups):
                    nc.tensor.matmul(sp[:, 2 * t:2 * t + 2],
                                     lhsT=HT[buf][:, t * P:(t + 1) * P],
                                     rhs=z2[:], start=True, stop=True)
                scr = ro.tile([P, 2 * groups], F32, tag="scr")
                nc.vector.tensor_scalar_add(scr[:], sp[:], bb_sb[:, 0:1])
                nc.sync.dma_start(scores_d[:], scr[:])

            # ---------------- pipelined schedule ----------------
            if level < 1:
                for i in range(repeat):
                    phase1(i % 2)
                _early_out(nc, tc, scores_d, groups)
            elif level < 2:
                for i in range(repeat):
                    phase1(i % 2)
                    allgather(i % 2)
                _early_out(nc, tc, scores_d, groups)
            else:
                phase1(0)
                allgather(0)
                for i in range(repeat):
                    cur, nxt = i % 2, (i + 1) % 2
                    if i + 1 < repeat:
                        phase1(nxt)
                    phase2(cur, nxt, do_ag_mid=(i + 1 < repeat))
                    if level >= 4:
                        allreduce(cur)
                        phase3(cur)
                if level < 4:
                    _early_out(nc, tc, scores_d, groups)

    nc.compile()
    return nc


class _EarlyStop(Exception):
    pass


def _early_out(nc, tc, scores_d, groups):
    with tc.tile_pool(name="eo", bufs=1) as eo:
        scr = eo.tile([P, 2 * groups], F32)
        nc.vector.memset(scr[:], 0.0)
        nc.sync.dma_start(scores_d[:], scr[:])


# --------------------------------------------------------------------------
# host glue
# --------------------------------------------------------------------------

def _make_in_maps(pl, inputs):
    ncores = pl.ncores
    pos, neg = inputs["pos"], inputs["neg"]
    local_n, local_pad = pl.local_n, pl.local_pad
    a_val = np.float32(np.asarray(inputs["prelu_a"]).reshape(-1)[0])
    bb_val = np.float32(np.asarray(inputs["b_bil"]).reshape(-1)[0])
    w2 = np.ascontiguousarray(
        np.asarray(inputs["W_gcn"]).T.astype(ml_dtypes.bfloat16))
    wbt = np.ascontiguousarray(np.asarray(inputs["W_bil"]).T.astype(np.float32))
    bgv = np.asarray(inputs["b_gcn"]).reshape(-1).astype(np.float32)
    bias2 = np.tile(np.concatenate([bgv, bgv])[None, :], (P, 1)).astype(
        ml_dtypes.bfloat16)

    posT = np.asarray(pos[0]).T.astype(ml_dtypes.bfloat16)   # [128, N]
    negT = np.asarray(neg[0]).T.astype(ml_dtypes.bfloat16)

    in_maps = []
    for c in range(ncores):
        sl = slice(c * local_n, (c + 1) * local_n)
        x2 = np.zeros((P, 2 * local_pad), ml_dtypes.bfloat16)
        x2[:, :local_n] = posT[:, sl]
        x2[:, local_pad:local_pad + local_n] = negT[:, sl]
        in_maps.append({
            "x2": x2,
            "mbh": pl.mbh[c],
            "trl": pl.trl[c],
            "tvv": pl.tvv[c],
            "iot": np.tile(np.arange(QUAD * P, dtype=np.float32)[None, :],
                           (P, 1)),
            "w2": w2,
            "bias2": bias2,
            "wbt": wbt,
            "acol": np.full((P, 1), a_val, np.float32),
            "bbcol": np.full((P, 1), bb_val, np.float32),
            "gidx": pl.gidx[c],
        })
    return in_maps


def _assemble(pl, results, n_total):
    ncores, local_n, local_pad = pl.ncores, pl.local_n, pl.local_pad
    logits = np.zeros((1, 2 * n_total), np.float32)
    for c in range(ncores):
        arr = results[c]["scores"]            # [P, 2*groups]
        posv = arr[:, 0::2].T.reshape(local_pad)[:local_n]
        negv = arr[:, 1::2].T.reshape(local_pad)[:local_n]
        logits[0, c * local_n:(c + 1) * local_n] = posv
        logits[0, n_total + c * local_n:n_total + (c + 1) * local_n] = negv
    return logits


def _run(pos, neg, edge_rows, edge_cols, edge_vals,
         W_gcn, b_gcn, prelu_a, W_bil, b_bil, ncores=8, **run_kwargs):
    n_nodes = pos.shape[1]
    f_dim = pos.shape[2]
    assert f_dim == P

    pl = make_plan(n_nodes, ncores, edge_rows, edge_cols, edge_vals)

    key = (n_nodes, ncores, pl.b_total, pl.gidx_cols, pl.mbh_cols)
    if key in _NC_CACHE:
        nc = _NC_CACHE[key]
    else:
        nc = build_nc(pl)
        _NC_CACHE.clear()
        _NC_CACHE[key] = nc

    in_maps = _make_in_maps(pl, {
        "pos": pos, "neg": neg, "W_gcn": W_gcn, "b_gcn": b_gcn,
        "prelu_a": prelu_a, "W_bil": W_bil, "b_bil": b_bil,
    })

    res = run_bass_kernel_spmd(nc, in_maps, core_ids=list(range(ncores)),
                               **run_kwargs)

    logits = _assemble(pl, res.results, n_nodes)
    return logits, res


def kernel(pos, neg, edge_rows, edge_cols, edge_vals,
           W_gcn, b_gcn, prelu_a, W_bil, b_bil):
    logits, _ = _run(pos, neg, edge_rows, edge_cols, edge_vals,
                     W_gcn, b_gcn, prelu_a, W_bil, b_bil)
    return logits
